# revision 10
# baseline (speedup 1.0000x reference)
"""Trainium2 Bass kernel for nn_Cate2Classifier (SWEM text classifier).

Strategy: data-parallel over batch across 8 NeuronCores (32 rows/core).
Per core the device does:
  - indirect-DMA gather of all 8000 token embeddings (title+desc) from a
    host-compacted per-core embedding table (unique tokens + sentinel row),
  - avg-pool via segment matmul on PE (validity and 1/len folded into
    host-built weights), max-pool via DVE tensor_max accumulation followed
    by PE transpose + free-dim reduce_max,
  - fc matmul producing h.T [1024, 32] in PSUM,
  - BatchNorm batch statistics with an 8KB cross-core AllReduce,
  - normalize+ReLU fused on ACT, classifier matmul, mask apply, store.
"""

import numpy as np

B, LT, LD = 256, 50, 200
V, D, H = 100000, 512, 1024
C1, C2 = 10, 64
NCORES = 8
BS = B // NCORES          # 32 batch rows per core
LANES = 4                 # partitions per batch row in a gather tile
LTP = 52                  # title length padded to LANES multiple
TT = LTP // LANES         # 13 title tiles
DTT = LD // LANES         # 50 desc tiles
NT = TT + DTT             # 63 gather tiles of [128, 512]
CAP = 8000                # max unique tokens per core (32*250)
SENT = CAP                # local sentinel row id (holds -1e30)
BN_EPS = 1e-5
NEG = -1.0e30

_CACHE = {}


def _build_nc():
    import concourse.bass as bass
    import concourse.bacc as bacc
    import concourse.tile as tile
    from concourse import mybir
    from concourse.bass import IndirectOffsetOnAxis
    from concourse.masks import make_identity

    f32 = mybir.dt.float32
    i32 = mybir.dt.int32
    X = mybir.AxisListType.X
    AF = mybir.ActivationFunctionType

    nc = bacc.Bacc("TRN2", num_devices=NCORES)

    emb_d = nc.dram_tensor("emb", [CAP + 1, D], f32, kind="ExternalInput")
    idx_d = nc.dram_tensor("idx", [128, NT], i32, kind="ExternalInput")
    w_d = nc.dram_tensor("wavg", [128, NT, BS], f32, kind="ExternalInput")
    fcw_d = nc.dram_tensor("fcw", [128, 16, H], f32, kind="ExternalInput")
    clfw_d = nc.dram_tensor("clfw", [128, 8, C2], f32, kind="ExternalInput")
    gb_d = nc.dram_tensor("gbeta", [128, 16], f32, kind="ExternalInput")
    mask_d = nc.dram_tensor("maskab", [C2, 2, BS], f32, kind="ExternalInput")
    out_d = nc.dram_tensor("out", [C2, BS], f32, kind="ExternalOutput")

    with tile.TileContext(nc) as tc:
        with (
            tc.tile_pool(name="const", bufs=1) as const,
            tc.tile_pool(name="gpool", bufs=6) as gpool,
            tc.tile_pool(name="work", bufs=1) as work,
            tc.tile_pool(name="pst", bufs=1, space="PSUM") as pst,
            tc.tile_pool(name="psacc", bufs=1, space="PSUM") as psacc,
            tc.tile_pool(name="dram", bufs=1, space="DRAM") as dram,
        ):
            idx_sb = const.tile([128, NT], i32)
            nc.sync.dma_start(idx_sb[:], idx_d[:])
            w_sb = const.tile([128, NT, BS], f32)
            nc.sync.dma_start(w_sb[:], w_d[:])
            fcw_sb = const.tile([128, 16, H], f32)
            nc.sync.dma_start(fcw_sb[:], fcw_d[:])
            clfw_sb = const.tile([128, 8, C2], f32)
            nc.sync.dma_start(clfw_sb[:], clfw_d[:])
            gb_sb = const.tile([128, 16], f32)
            nc.sync.dma_start(gb_sb[:], gb_d[:])
            mask_sb = const.tile([C2, 2, BS], f32)
            nc.sync.dma_start(mask_sb[:], mask_d[:])
            ident = const.tile([128, 128], f32)
            make_identity(nc, ident[:])

            # PE pre-touch: the PE LoadWeights micro-op supports only one
            # sync-wait, so let PE observe each weight tensor's DMA (and
            # gpsimd's identity) via throwaway matmuls before the real ones.
            ps_o = psacc.tile([C2, BS], f32)
            for pre in (ident[:, 0:BS], w_sb[:, 0, :], fcw_sb[:, 0, 0:BS],
                        clfw_sb[:, 0, 0:BS]):
                nc.tensor.matmul(
                    ps_o[0:BS, 0:1], lhsT=pre, rhs=ident[:, 0:1],
                    start=True, stop=True,
                )

            # max-pool accumulators (token lanes on partitions)
            acc_t = work.tile([128, D], f32)
            acc_d = work.tile([128, D], f32)
            # avg-pool psum accumulators (batch rows on partitions)
            ps_avg_t = psacc.tile([BS, D], f32)
            ps_avg_d = psacc.tile([BS, D], f32)

            for j in range(NT):
                g = gpool.tile([128, D], f32, tag="g")
                nc.gpsimd.indirect_dma_start(
                    out=g[:],
                    out_offset=None,
                    in_=emb_d[:, :],
                    in_offset=IndirectOffsetOnAxis(ap=idx_sb[:, j : j + 1], axis=0),
                )
                if j < TT:
                    ps, jj, jlast, acc = ps_avg_t, j, TT - 1, acc_t
                else:
                    ps, jj, jlast, acc = ps_avg_d, j - TT, DTT - 1, acc_d
                nc.tensor.matmul(
                    ps[:],
                    lhsT=w_sb[:, j, :],
                    rhs=g[:],
                    start=(jj == 0),
                    stop=(jj == jlast),
                )
                if jj == 0:
                    nc.vector.tensor_copy(acc[:], g[:])
                else:
                    nc.vector.tensor_max(acc[:], acc[:], g[:])

            avg_t_sb = work.tile([BS, D], f32)
            nc.vector.tensor_copy(avg_t_sb[:], ps_avg_t[:])
            avg_d_sb = work.tile([BS, D], f32)
            nc.vector.tensor_copy(avg_d_sb[:], ps_avg_d[:])

            # swemT layout: [128, 16, BS]; chunk c holds features 128c..128c+127
            # of swem = [avgT | maxT | avgD | maxD]
            swemT = work.tile([128, 16, BS], f32)
            t_avg = pst.tile([128, 8, BS], f32)
            t_max_t = pst.tile([128, 4, 128], f32)
            t_max_d = pst.tile([128, 4, 128], f32)
            for c in range(4):
                nc.tensor.transpose(
                    t_avg[:, c, :], avg_t_sb[:, c * 128 : (c + 1) * 128],
                    ident[:BS, :BS]
                )
                nc.vector.tensor_copy(swemT[:, c, :], t_avg[:, c, :])
            for c in range(4):
                nc.tensor.transpose(
                    t_max_t[:, c, :], acc_t[:, c * 128 : (c + 1) * 128], ident[:]
                )
                nc.vector.reduce_max(
                    swemT[:, 4 + c, :],
                    t_max_t[:, c, :].rearrange("p (r l) -> p r l", l=LANES),
                    axis=X,
                )
            for c in range(4):
                nc.tensor.transpose(
                    t_avg[:, 4 + c, :], avg_d_sb[:, c * 128 : (c + 1) * 128],
                    ident[:BS, :BS]
                )
                nc.vector.tensor_copy(swemT[:, 8 + c, :], t_avg[:, 4 + c, :])
            for c in range(4):
                nc.tensor.transpose(
                    t_max_d[:, c, :], acc_d[:, c * 128 : (c + 1) * 128], ident[:]
                )
                nc.vector.reduce_max(
                    swemT[:, 12 + c, :],
                    t_max_d[:, c, :].rearrange("p (r l) -> p r l", l=LANES),
                    axis=X,
                )

            # h.T = fc_w.T @ swem.T accumulated over 16 K-chunks -> [128, 8, BS]
            ps_h = psacc.tile([128, 8, BS], f32)
            for m in range(8):
                for k in range(16):
                    nc.tensor.matmul(
                        ps_h[:, m, :],
                        lhsT=fcw_sb[:, k, m * 128 : (m + 1) * 128],
                        rhs=swemT[:, k, :],
                        start=(k == 0),
                        stop=(k == 15),
                    )

            # BatchNorm statistics: per-feature sum and sum-of-squares over
            # the local 32 batch rows, then AllReduce across cores.
            stats = work.tile([128, 16], f32)
            nc.vector.reduce_sum(stats[:, 0:8], ps_h[:], axis=X)
            hsq = work.tile([128, 8, BS], f32)
            nc.scalar.square(hsq[:], ps_h[:])
            nc.vector.reduce_sum(stats[:, 8:16], hsq[:], axis=X)

            cc_in = dram.tile([128, 16], f32)
            cc_out = dram.tile([128, 16], f32)
            nc.sync.dma_start(cc_in[:], stats[:])
            nc.gpsimd.collective_compute(
                "AllReduce",
                mybir.AluOpType.add,
                replica_groups=[list(range(NCORES))],
                ins=[cc_in.opt()],
                outs=[cc_out.opt()],
            )
            statg = work.tile([128, 16], f32)
            nc.sync.dma_start(statg[:], cc_out[:])

            mean = work.tile([128, 8], f32)
            nc.vector.tensor_scalar_mul(mean[:], statg[:, 0:8], 1.0 / B)
            ex2 = work.tile([128, 8], f32)
            nc.vector.tensor_scalar_mul(ex2[:], statg[:, 8:16], 1.0 / B)
            var = work.tile([128, 8], f32)
            nc.vector.tensor_mul(var[:], mean[:], mean[:])
            nc.vector.tensor_sub(var[:], ex2[:], var[:])
            eps_sb = work.tile([128, 1], f32)
            nc.vector.memset(eps_sb[:], BN_EPS)
            sdev = work.tile([128, 8], f32)
            nc.scalar.activation(
                sdev[:], var[:], AF.Sqrt, bias=eps_sb[:, 0:1]
            )
            rstd = work.tile([128, 8], f32)
            nc.vector.reciprocal(rstd[:], sdev[:])
            avec = work.tile([128, 8], f32)
            nc.vector.tensor_mul(avec[:], gb_sb[:, 0:8], rstd[:])
            bvec = work.tile([128, 8], f32)
            nc.vector.tensor_mul(bvec[:], mean[:], avec[:])
            nc.vector.tensor_sub(bvec[:], gb_sb[:, 8:16], bvec[:])

            # normalize + ReLU, fused per 128-feature chunk
            hn = work.tile([128, 8, BS], f32)
            for m in range(8):
                nc.scalar.activation(
                    hn[:, m, :],
                    ps_h[:, m, :],
                    AF.Relu,
                    bias=bvec[:, m : m + 1],
                    scale=avec[:, m : m + 1],
                )

            # classifier: out.T [64, 32] = clf_w.T @ hn
            for k in range(8):
                nc.tensor.matmul(
                    ps_o[:],
                    lhsT=clfw_sb[:, k, :],
                    rhs=hn[:, k, :],
                    start=(k == 0),
                    stop=(k == 7),
                )
            o_sb = work.tile([C2, BS], f32)
            nc.vector.tensor_mul(o_sb[:], ps_o[:], mask_sb[:, 0, :])
            nc.vector.tensor_add(o_sb[:], o_sb[:], mask_sb[:, 1, :])
            nc.sync.dma_start(out_d[:], o_sb[:])

    nc.finalize()
    return nc


def _lane_tiles(a):
    """[BS, ntiles*LANES] -> [ntiles, 128] with partition p = 4*row + lane."""
    bs, total = a.shape
    nt = total // LANES
    return np.ascontiguousarray(
        a.reshape(bs, nt, LANES).transpose(1, 0, 2).reshape(nt, bs * LANES)
    )


def _prep_core(title, desc, t_len, d_len, cate1, mask1, emb, clf_b):
    # mark invalid token slots with -1
    tok_t = np.where(np.arange(LT)[None, :] < t_len[:, None], title, -1)
    tok_t = np.concatenate(
        [tok_t, np.full((BS, LTP - LT), -1, dtype=tok_t.dtype)], axis=1
    )
    tok_d = np.where(np.arange(LD)[None, :] < d_len[:, None], desc, -1)

    both = np.concatenate([tok_t.ravel(), tok_d.ravel()])
    uniq = np.unique(both[both >= 0])
    emb_core = np.zeros((CAP + 1, D), dtype=np.float32)
    emb_core[: uniq.size] = emb[uniq]
    emb_core[SENT] = NEG

    def remap(tok):
        loc = np.searchsorted(uniq, np.clip(tok, 0, None)).astype(np.int32)
        return np.where(tok >= 0, loc, np.int32(SENT)).astype(np.int32)

    idx_tiles = np.concatenate(
        [_lane_tiles(remap(tok_t)), _lane_tiles(remap(tok_d))], axis=0
    )  # [63, 128]
    idxT = np.ascontiguousarray(idx_tiles.T).astype(np.int32)  # [128, 63]

    wt_t = (
        (tok_t >= 0).astype(np.float32)
        / np.maximum(t_len, 1).astype(np.float32)[:, None]
    )
    wt_d = (
        (tok_d >= 0).astype(np.float32)
        / np.maximum(d_len, 1).astype(np.float32)[:, None]
    )
    wvals = np.concatenate([_lane_tiles(wt_t), _lane_tiles(wt_d)], axis=0)  # [63,128]
    W = np.zeros((NT, 128, BS), dtype=np.float32)
    W[:, np.arange(128), np.repeat(np.arange(BS), LANES)] = wvals
    w_host = np.ascontiguousarray(W.transpose(1, 0, 2))  # [128, 63, 32]

    m = mask1[cate1]  # [32, 64] bool
    A = (~m).T.astype(np.float32)  # [64, 32]
    Bp = A * clf_b.astype(np.float32)[:, None] + (-100.0) * m.T.astype(np.float32)
    maskab = np.ascontiguousarray(np.stack([A, Bp], axis=1))  # [64, 2, 32]

    return emb_core, idxT, w_host, maskab


def kernel(**inputs):
    title = np.asarray(inputs["title"]).astype(np.int64)
    desc = np.asarray(inputs["desc"]).astype(np.int64)
    t_len = np.asarray(inputs["t_len"]).astype(np.int64)
    d_len = np.asarray(inputs["d_len"]).astype(np.int64)
    cate1 = np.asarray(inputs["cate1"]).astype(np.int64)
    mask1 = np.asarray(inputs["mask1"]).astype(bool)
    emb = np.asarray(inputs["emb"]).astype(np.float32)
    fc_w = np.asarray(inputs["fc_w"]).astype(np.float32)
    gamma = np.asarray(inputs["gamma"]).astype(np.float32)
    beta = np.asarray(inputs["beta"]).astype(np.float32)
    clf_w = np.asarray(inputs["clf_w"]).astype(np.float32)
    clf_b = np.asarray(inputs["clf_b"]).astype(np.float32)
    # fc_b cancels exactly in training-mode BatchNorm (constant per-feature
    # shift is removed by mean subtraction), so it is unused.

    if "nc" not in _CACHE:
        _CACHE["nc"] = _build_nc()
    nc = _CACHE["nc"]

    fcw_host = np.ascontiguousarray(fc_w.reshape(16, 128, H).transpose(1, 0, 2))
    clfw_host = np.ascontiguousarray(clf_w.reshape(8, 128, C2).transpose(1, 0, 2))
    gb_host = np.ascontiguousarray(
        np.concatenate([gamma.reshape(8, 128).T, beta.reshape(8, 128).T], axis=1)
    ).astype(np.float32)

    in_maps = []
    for c in range(NCORES):
        rows = slice(c * BS, (c + 1) * BS)
        emb_core, idxT, w_host, maskab = _prep_core(
            title[rows], desc[rows], t_len[rows], d_len[rows], cate1[rows],
            mask1, emb, clf_b,
        )
        in_maps.append(
            {
                "emb": emb_core,
                "idx": idxT,
                "wavg": w_host,
                "fcw": fcw_host,
                "clfw": clfw_host,
                "gbeta": gb_host,
                "maskab": maskab,
            }
        )

    from concourse.bass_utils import run_bass_kernel_spmd

    res = run_bass_kernel_spmd(nc, in_maps, core_ids=list(range(NCORES)))
    _CACHE["last_result"] = res
    outs = [r["out"] for r in res.results]  # each [64, 32]
    full = np.concatenate([np.asarray(o).T for o in outs], axis=0)  # [256, 64]
    return np.ascontiguousarray(full.astype(np.float32))


# revision 15
# speedup vs baseline: 1.1839x; 1.1839x over previous
"""Trainium2 Bass kernel for nn_Cate2Classifier (SWEM text classifier).

Strategy: data-parallel over batch across 8 NeuronCores (32 rows/core).
Per core the device does:
  - indirect-DMA gather of all 8000 token embeddings (title+desc) from a
    host-compacted per-core embedding table (unique tokens + sentinel row),
  - avg-pool via segment matmul on PE (validity and 1/len folded into
    host-built weights), max-pool via DVE tensor_max accumulation followed
    by PE transpose + free-dim reduce_max,
  - fc matmul producing h.T [1024, 32] in PSUM,
  - BatchNorm batch statistics with an 8KB cross-core AllReduce,
  - normalize+ReLU fused on ACT, classifier matmul, mask apply, store.
"""

import ml_dtypes
import numpy as np

BF16 = ml_dtypes.bfloat16

B, LT, LD = 256, 50, 200
V, D, H = 100000, 512, 1024
C1, C2 = 10, 64
NCORES = 8
BS = B // NCORES          # 32 batch rows per core
LANES = 4                 # partitions per batch row in a gather tile
LTP = 52                  # title length padded to LANES multiple
TT = LTP // LANES         # 13 title tiles
DTT = LD // LANES         # 50 desc tiles
NT = TT + DTT             # 63 gather tiles of [128, 512]
CAP = 8000                # max unique tokens per core (32*250)
SENT = CAP                # local sentinel row id (holds -1e30)
BN_EPS = 1e-5
NEG = -1.0e30

_CACHE = {}


def _build_nc():
    import concourse.bass as bass
    import concourse.bacc as bacc
    import concourse.tile as tile
    from concourse import mybir
    from concourse.bass import IndirectOffsetOnAxis
    from concourse.masks import make_identity

    f32 = mybir.dt.float32
    bf16 = mybir.dt.bfloat16
    i32 = mybir.dt.int32
    X = mybir.AxisListType.X
    AF = mybir.ActivationFunctionType

    nc = bacc.Bacc("TRN2", num_devices=NCORES)

    emb_d = nc.dram_tensor("emb", [CAP + 1, D], bf16, kind="ExternalInput")
    idx_d = nc.dram_tensor("idx", [128, NT], i32, kind="ExternalInput")
    w_d = nc.dram_tensor("wavg", [128, NT, BS], bf16, kind="ExternalInput")
    lrec_d = nc.dram_tensor("lrec", [BS, 2], f32, kind="ExternalInput")
    fcw_d = nc.dram_tensor("fcw", [128, 16, H], bf16, kind="ExternalInput")
    clfw_d = nc.dram_tensor("clfw", [128, 8, C2], bf16, kind="ExternalInput")
    gb_d = nc.dram_tensor("gbeta", [128, 16], f32, kind="ExternalInput")
    mask_d = nc.dram_tensor("maskab", [C2, 2, BS], f32, kind="ExternalInput")
    out_d = nc.dram_tensor("out", [C2, BS], f32, kind="ExternalOutput")

    with tile.TileContext(nc) as tc:
        with (
            tc.tile_pool(name="const", bufs=1) as const,
            tc.tile_pool(name="gpool", bufs=1) as gpool,
            tc.tile_pool(name="work", bufs=1) as work,
            tc.tile_pool(name="pst", bufs=1, space="PSUM") as pst,
            tc.tile_pool(name="psacc", bufs=1, space="PSUM") as psacc,
            tc.tile_pool(name="dram", bufs=1, space="DRAM") as dram,
        ):
            idx_sb = const.tile([128, NT], i32)
            nc.sync.dma_start(idx_sb[:], idx_d[:])
            w_sb = const.tile([128, NT, BS], bf16)
            nc.sync.dma_start(w_sb[:], w_d[:])
            lrec_sb = const.tile([BS, 2], f32)
            nc.sync.dma_start(lrec_sb[:], lrec_d[:])
            fcw_sb = const.tile([128, 16, H], bf16)
            nc.sync.dma_start(fcw_sb[:], fcw_d[:])
            clfw_sb = const.tile([128, 8, C2], bf16)
            nc.sync.dma_start(clfw_sb[:], clfw_d[:])
            gb_sb = const.tile([128, 16], f32)
            nc.sync.dma_start(gb_sb[:], gb_d[:])
            mask_sb = const.tile([C2, 2, BS], f32)
            nc.sync.dma_start(mask_sb[:], mask_d[:])
            ident = const.tile([128, 128], bf16)
            make_identity(nc, ident[:])

            # PE pre-touch: the PE LoadWeights micro-op supports only one
            # sync-wait, so let PE observe each weight tensor's DMA (and
            # gpsimd's identity) via throwaway matmuls before the real ones.
            ps_o = psacc.tile([C2, BS], f32)
            for pre in (ident[:, 0:BS], w_sb[:, 0, :], fcw_sb[:, 0, 0:BS],
                        clfw_sb[:, 0, 0:BS]):
                nc.tensor.matmul(
                    ps_o[0:BS, 0:1], lhsT=pre, rhs=ident[:, 0:1],
                    start=True, stop=True,
                )

            # max-pool accumulators (token lanes on partitions)
            acc_t = work.tile([128, D], bf16)
            acc_d = work.tile([128, D], bf16)
            # avg-pool psum accumulators (batch rows on partitions)
            ps_avg_t = psacc.tile([BS, D], f32)
            ps_avg_d = psacc.tile([BS, D], f32)

            gts = []
            for b in range(NT):
                g = gpool.tile([128, D], bf16, tag=f"g{b}")
                nc.gpsimd.indirect_dma_start(
                    out=g[:],
                    out_offset=None,
                    in_=emb_d[:, :],
                    in_offset=IndirectOffsetOnAxis(
                        ap=idx_sb[:, b : b + 1], axis=0
                    ),
                )
                gts.append(g)
            for j in range(NT):
                g = gts[j][:, :]
                if j < TT:
                    ps, jj, jlast, acc = ps_avg_t, j, TT - 1, acc_t
                else:
                    ps, jj, jlast, acc = ps_avg_d, j - TT, DTT - 1, acc_d
                nc.tensor.matmul(
                    ps[:],
                    lhsT=w_sb[:, j, :],
                    rhs=g,
                    start=(jj == 0),
                    stop=(jj == jlast),
                )
                if jj == 0:
                    nc.vector.tensor_copy(acc[:], g)
                else:
                    nc.vector.tensor_max(acc[:], acc[:], g)

            avg_t_sb = work.tile([BS, D], bf16)
            nc.vector.tensor_scalar_mul(avg_t_sb[:], ps_avg_t[:], lrec_sb[:, 0:1])
            avg_d_sb = work.tile([BS, D], bf16)
            nc.vector.tensor_scalar_mul(avg_d_sb[:], ps_avg_d[:], lrec_sb[:, 1:2])

            # swemT layout: [128, 16, BS]; chunk c holds features 128c..128c+127
            # of swem = [avgT | maxT | avgD | maxD]
            swemT = work.tile([128, 16, BS], bf16)
            t_avg = pst.tile([128, 8, BS], bf16)
            t_max_t = pst.tile([128, 4, 128], bf16)
            t_max_d = pst.tile([128, 4, 128], bf16)
            for c in range(4):
                nc.tensor.transpose(
                    t_avg[:, c, :], avg_t_sb[:, c * 128 : (c + 1) * 128],
                    ident[:BS, :BS]
                )
                nc.vector.tensor_copy(swemT[:, c, :], t_avg[:, c, :])
            for c in range(4):
                nc.tensor.transpose(
                    t_max_t[:, c, :], acc_t[:, c * 128 : (c + 1) * 128], ident[:]
                )
                nc.vector.reduce_max(
                    swemT[:, 4 + c, :],
                    t_max_t[:, c, :].rearrange("p (r l) -> p r l", l=LANES),
                    axis=X,
                )
            for c in range(4):
                nc.tensor.transpose(
                    t_avg[:, 4 + c, :], avg_d_sb[:, c * 128 : (c + 1) * 128],
                    ident[:BS, :BS]
                )
                nc.vector.tensor_copy(swemT[:, 8 + c, :], t_avg[:, 4 + c, :])
            for c in range(4):
                nc.tensor.transpose(
                    t_max_d[:, c, :], acc_d[:, c * 128 : (c + 1) * 128], ident[:]
                )
                nc.vector.reduce_max(
                    swemT[:, 12 + c, :],
                    t_max_d[:, c, :].rearrange("p (r l) -> p r l", l=LANES),
                    axis=X,
                )

            # h.T = fc_w.T @ swem.T accumulated over 16 K-chunks -> [128, 8, BS]
            ps_h = psacc.tile([128, 8, BS], f32)
            for m in range(8):
                for k in range(16):
                    nc.tensor.matmul(
                        ps_h[:, m, :],
                        lhsT=fcw_sb[:, k, m * 128 : (m + 1) * 128],
                        rhs=swemT[:, k, :],
                        start=(k == 0),
                        stop=(k == 15),
                    )

            # BatchNorm statistics: per-feature sum and sum-of-squares over
            # the local 32 batch rows, then AllReduce across cores.
            stats = work.tile([128, 16], f32)
            nc.vector.reduce_sum(stats[:, 0:8], ps_h[:], axis=X)
            hsq = work.tile([128, 8, BS], f32)
            nc.scalar.square(hsq[:], ps_h[:])
            nc.vector.reduce_sum(stats[:, 8:16], hsq[:], axis=X)

            cc_in = dram.tile([128, 16], f32)
            cc_out = dram.tile([128, 16], f32)
            nc.sync.dma_start(cc_in[:], stats[:])
            nc.gpsimd.collective_compute(
                "AllReduce",
                mybir.AluOpType.add,
                replica_groups=[list(range(NCORES))],
                ins=[cc_in.opt()],
                outs=[cc_out.opt()],
            )
            statg = work.tile([128, 16], f32)
            nc.sync.dma_start(statg[:], cc_out[:])

            mean = work.tile([128, 8], f32)
            nc.vector.tensor_scalar_mul(mean[:], statg[:, 0:8], 1.0 / B)
            ex2 = work.tile([128, 8], f32)
            nc.vector.tensor_scalar_mul(ex2[:], statg[:, 8:16], 1.0 / B)
            var = work.tile([128, 8], f32)
            nc.vector.tensor_mul(var[:], mean[:], mean[:])
            nc.vector.tensor_sub(var[:], ex2[:], var[:])
            eps_sb = work.tile([128, 1], f32)
            nc.vector.memset(eps_sb[:], BN_EPS)
            sdev = work.tile([128, 8], f32)
            nc.scalar.activation(
                sdev[:], var[:], AF.Sqrt, bias=eps_sb[:, 0:1]
            )
            rstd = work.tile([128, 8], f32)
            nc.vector.reciprocal(rstd[:], sdev[:])
            avec = work.tile([128, 8], f32)
            nc.vector.tensor_mul(avec[:], gb_sb[:, 0:8], rstd[:])
            bvec = work.tile([128, 8], f32)
            nc.vector.tensor_mul(bvec[:], mean[:], avec[:])
            nc.vector.tensor_sub(bvec[:], gb_sb[:, 8:16], bvec[:])

            # normalize + ReLU, fused per 128-feature chunk
            hn = work.tile([128, 8, BS], bf16)
            for m in range(8):
                nc.scalar.activation(
                    hn[:, m, :],
                    ps_h[:, m, :],
                    AF.Relu,
                    bias=bvec[:, m : m + 1],
                    scale=avec[:, m : m + 1],
                )

            # classifier: out.T [64, 32] = clf_w.T @ hn
            for k in range(8):
                nc.tensor.matmul(
                    ps_o[:],
                    lhsT=clfw_sb[:, k, :],
                    rhs=hn[:, k, :],
                    start=(k == 0),
                    stop=(k == 7),
                )
            o_sb = work.tile([C2, BS], f32)
            nc.vector.tensor_mul(o_sb[:], ps_o[:], mask_sb[:, 0, :])
            nc.vector.tensor_add(o_sb[:], o_sb[:], mask_sb[:, 1, :])
            nc.sync.dma_start(out_d[:], o_sb[:])

    nc.finalize()
    return nc


def _lane_tiles(a):
    """[BS, ntiles*LANES] -> [ntiles, 128] with partition p = 4*row + lane."""
    bs, total = a.shape
    nt = total // LANES
    return np.ascontiguousarray(
        a.reshape(bs, nt, LANES).transpose(1, 0, 2).reshape(nt, bs * LANES)
    )


def _prep_core(title, desc, t_len, d_len, cate1, mask1, emb, clf_b):
    # mark invalid token slots with -1
    tok_t = np.where(np.arange(LT)[None, :] < t_len[:, None], title, -1)
    tok_t = np.concatenate(
        [tok_t, np.full((BS, LTP - LT), -1, dtype=tok_t.dtype)], axis=1
    )
    tok_d = np.where(np.arange(LD)[None, :] < d_len[:, None], desc, -1)

    both = np.concatenate([tok_t.ravel(), tok_d.ravel()])
    uniq = np.unique(both[both >= 0])
    emb_core = np.zeros((CAP + 1, D), dtype=BF16)
    emb_core[: uniq.size] = emb[uniq].astype(BF16)
    emb_core[SENT] = BF16(NEG)

    def remap(tok):
        loc = np.searchsorted(uniq, np.clip(tok, 0, None)).astype(np.int32)
        return np.where(tok >= 0, loc, np.int32(SENT)).astype(np.int32)

    idx_tiles = np.concatenate(
        [_lane_tiles(remap(tok_t)), _lane_tiles(remap(tok_d))], axis=0
    )  # [63, 128]
    idxT = np.ascontiguousarray(idx_tiles.T).astype(np.int32)  # [128, 63]

    # 0/1 validity weights (exact in bf16); 1/len is applied on-device as a
    # per-row f32 tensor_scalar_mul via lrec.
    wt_t = (tok_t >= 0).astype(np.float32)
    wt_d = (tok_d >= 0).astype(np.float32)
    wvals = np.concatenate([_lane_tiles(wt_t), _lane_tiles(wt_d)], axis=0)  # [63,128]
    W = np.zeros((NT, 128, BS), dtype=np.float32)
    W[:, np.arange(128), np.repeat(np.arange(BS), LANES)] = wvals
    w_host = np.ascontiguousarray(W.transpose(1, 0, 2)).astype(BF16)  # [128,63,32]
    lrec = np.stack(
        [
            1.0 / np.maximum(t_len, 1).astype(np.float32),
            1.0 / np.maximum(d_len, 1).astype(np.float32),
        ],
        axis=1,
    ).astype(np.float32)  # [32, 2]

    m = mask1[cate1]  # [32, 64] bool
    A = (~m).T.astype(np.float32)  # [64, 32]
    Bp = A * clf_b.astype(np.float32)[:, None] + (-100.0) * m.T.astype(np.float32)
    maskab = np.ascontiguousarray(np.stack([A, Bp], axis=1))  # [64, 2, 32]

    return emb_core, idxT, w_host, lrec, maskab


def kernel(**inputs):
    title = np.asarray(inputs["title"]).astype(np.int64)
    desc = np.asarray(inputs["desc"]).astype(np.int64)
    t_len = np.asarray(inputs["t_len"]).astype(np.int64)
    d_len = np.asarray(inputs["d_len"]).astype(np.int64)
    cate1 = np.asarray(inputs["cate1"]).astype(np.int64)
    mask1 = np.asarray(inputs["mask1"]).astype(bool)
    emb = np.asarray(inputs["emb"]).astype(np.float32)
    fc_w = np.asarray(inputs["fc_w"]).astype(np.float32)
    gamma = np.asarray(inputs["gamma"]).astype(np.float32)
    beta = np.asarray(inputs["beta"]).astype(np.float32)
    clf_w = np.asarray(inputs["clf_w"]).astype(np.float32)
    clf_b = np.asarray(inputs["clf_b"]).astype(np.float32)
    # fc_b cancels exactly in training-mode BatchNorm (constant per-feature
    # shift is removed by mean subtraction), so it is unused.

    if "nc" not in _CACHE:
        _CACHE["nc"] = _build_nc()
    nc = _CACHE["nc"]

    fcw_host = np.ascontiguousarray(
        fc_w.reshape(16, 128, H).transpose(1, 0, 2)
    ).astype(BF16)
    clfw_host = np.ascontiguousarray(
        clf_w.reshape(8, 128, C2).transpose(1, 0, 2)
    ).astype(BF16)
    gb_host = np.ascontiguousarray(
        np.concatenate([gamma.reshape(8, 128).T, beta.reshape(8, 128).T], axis=1)
    ).astype(np.float32)

    in_maps = []
    for c in range(NCORES):
        rows = slice(c * BS, (c + 1) * BS)
        emb_core, idxT, w_host, lrec, maskab = _prep_core(
            title[rows], desc[rows], t_len[rows], d_len[rows], cate1[rows],
            mask1, emb, clf_b,
        )
        in_maps.append(
            {
                "emb": emb_core,
                "idx": idxT,
                "wavg": w_host,
                "lrec": lrec,
                "fcw": fcw_host,
                "clfw": clfw_host,
                "gbeta": gb_host,
                "maskab": maskab,
            }
        )

    from concourse.bass_utils import run_bass_kernel_spmd

    res = run_bass_kernel_spmd(nc, in_maps, core_ids=list(range(NCORES)))
    _CACHE["last_result"] = res
    outs = [r["out"] for r in res.results]  # each [64, 32]
    full = np.concatenate([np.asarray(o).T for o in outs], axis=0)  # [256, 64]
    return np.ascontiguousarray(full.astype(np.float32))
